# revision 17
# baseline (speedup 1.0000x reference)
"""Trainium2 Bass kernel for FeatureSimilarity (l2): out = -||f_i - f_j|| over all pairs.

Strategy (8 NeuronCores, SPMD):
  The distance matrix is symmetric, so computing the column slab
  out[:, c*1024:(c+1)*1024] on core c equals the row slab of the sharding
  hint while giving a layout where both norm terms can be folded into the
  matmul / activation with no extra per-element passes.

  Inputs per core: the transposed feature bank bankT [128, 8192] (full) and
  the core's transposed query slab qT [128, 1024] (a column slice of bankT).

  Device-side (per core):
    sq_n col [128, 64] = per-block PE reduce of bankT^2 (64 N=1 matmuls)
    sq_m row [1, 1024] = PE reduce of qT^2 (2 [1,512] matmuls, -0.5 scaled)
    per 128x1024 tile (64 of them):
      PSUM = bank_nb^T @ qT - 0.5*sq_m[j]      (2 fp32r matmuls + 2 K=1
             augmented fp32r matmuls; operands pre-rounded via DVE copies)
      SBUF = Sqrt(-2*PSUM + (sq_n[p] + EPS))   (ACT, bias AP)
      out  = SBUF * -1                         (DVE)
      DMA out
  EPS keeps the Sqrt input positive under fp/fp32r noise on the diagonal
  (off-diagonal d^2 >= ~70 for this data; error ~eps/(2*dist) < 1.5e-2 abs).
  The diagonal (exactly zero by definition) is written at gather time.
"""

import os
import sys

import numpy as np

sys.path.insert(0, "/opt/trn_rl_repo")

import concourse.bacc as bacc
import concourse.bass as bass
import concourse.mybir as mybir
import concourse.tile as tile
from concourse.bass_utils import run_bass_kernel_spmd

N = 8192
D = 128
NCORES = 8
S = N // NCORES  # 1024 columns per core
NB = N // 128  # 64 row blocks per core
EPS = 0.25  # added to d^2; keeps Sqrt input positive under fp/fp32r noise
F32 = mybir.dt.float32
F32R = mybir.dt.float32r

VARIANT = os.environ.get("KERNEL_VARIANT", "devsq2")

_STATE = {}
LAST_RESULTS = None


def _build_devsq2():
    """Device-side norms via PE reductions; fp32r matmuls with explicit
    rounding copies on DVE (BIR requires fp32r matmul operands to be
    produced rounded)."""
    nc = bacc.Bacc("TRN2", target_bir_lowering=False, debug=False, enable_asserts=False)

    bankT_d = nc.dram_tensor("bankT", [D, N], F32, kind="ExternalInput")
    qT_d = nc.dram_tensor("qT", [D, S], F32, kind="ExternalInput")
    out_d = nc.dram_tensor("out", [N, S], F32, kind="ExternalOutput")

    CH = 8
    CW = N // CH

    with tile.TileContext(nc) as tc:
        with (
            tc.tile_pool(name="persist", bufs=1) as persist,
            tc.tile_pool(name="psum", bufs=3, space=bass.MemorySpace.PSUM) as psum_pool,
            tc.tile_pool(name="prosum", bufs=1, space=bass.MemorySpace.PSUM) as prosum,
            tc.tile_pool(name="stage", bufs=3) as stage,
            tc.tile_pool(name="outp", bufs=3) as outp,
        ):
            qt = persist.tile([D, S], F32)
            qtr = persist.tile([D, S], F32R)
            nc.sync.dma_start(qt[:], qT_d.ap()[:])
            nc.vector.tensor_copy(qtr[:], qt[:])

            bank = persist.tile([D, N], F32)
            bankr = persist.tile([D, N], F32R)
            bsq = persist.tile([D, N], F32)
            qsq = persist.tile([D, S], F32)
            sqncol = persist.tile([128, NB], F32)  # sq_n + EPS, column form
            sqm = persist.tile([1, S], F32R)  # -0.5 * sq_m, row form (rounded)
            ones = persist.tile([1, 128], F32)
            onesr = persist.tile([1, 128], F32R)  # aug lhsT (rounded)
            onescol = persist.tile([128, 1], F32)  # rhs for sq_n reduce
            neghalf = persist.tile([128, 1], F32)  # lhsT for sq_m reduce
            nc.vector.memset(ones[:], 1.0)
            nc.vector.memset(onescol[:], 1.0)
            nc.vector.memset(neghalf[:], -0.5)
            nc.vector.tensor_copy(onesr[:], ones[:])

            # query norms: qsq = qt^2; sqm[0,j] = -0.5 * sum_d qsq[d,j]
            nc.vector.tensor_tensor(qsq[:], qt[:], qt[:], mybir.AluOpType.mult)
            pm = prosum.tile([1, 512], F32, tag="pro")
            for j in range(2):
                nc.tensor.matmul(
                    pm[:],
                    neghalf[:],
                    qsq[:, j * 512 : (j + 1) * 512],
                    start=True,
                    stop=True,
                )
                nc.vector.tensor_copy(sqm[:, j * 512 : (j + 1) * 512], pm[:])

            # bank norms, chunked with the bank DMA; rounding copy for matmuls
            pn = prosum.tile([128, NB], F32, tag="pro2")
            for k in range(CH):
                cs = slice(k * CW, (k + 1) * CW)
                nc.sync.dma_start(bank[:, cs], bankT_d.ap()[:, cs])
                nc.vector.tensor_copy(bankr[:, cs], bank[:, cs])
                nc.vector.tensor_tensor(
                    bsq[:, cs], bank[:, cs], bank[:, cs], mybir.AluOpType.mult
                )
                for b in range(CH):
                    col = k * CH + b
                    nc.tensor.matmul(
                        pn[:, col : col + 1],
                        bsq[:, col * 128 : (col + 1) * 128],
                        onescol[:],
                        start=True,
                        stop=True,
                    )
                nc.vector.tensor_scalar_add(
                    sqncol[:, k * CH : (k + 1) * CH],
                    pn[:, k * CH : (k + 1) * CH],
                    float(EPS),
                )

            for nb in range(NB):
                ps = psum_pool.tile([128, S], F32)
                for j in range(2):
                    nc.tensor.matmul(
                        ps[:, j * 512 : (j + 1) * 512],
                        bankr[:, nb * 128 : (nb + 1) * 128],
                        qtr[:, j * 512 : (j + 1) * 512],
                        start=True,
                        stop=False,
                    )
                for j in range(2):
                    nc.tensor.matmul(
                        ps[:, j * 512 : (j + 1) * 512],
                        onesr[:],
                        sqm[:, j * 512 : (j + 1) * 512],
                        start=False,
                        stop=True,
                    )
                st = stage.tile([128, S], F32)
                nc.scalar.activation(
                    st[:],
                    ps[:],
                    mybir.ActivationFunctionType.Sqrt,
                    bias=sqncol[:, nb : nb + 1],
                    scale=-2.0,
                )
                ot = outp.tile([128, S], F32)
                nc.vector.tensor_scalar_mul(ot[:], st[:], -1.0)
                nc.sync.dma_start(out_d.ap()[nb * 128 : (nb + 1) * 128, :], ot[:])

    nc.compile()
    return nc


def _build_hostsq():
    """v0: norms computed on host and passed as inputs."""
    nc = bacc.Bacc("TRN2", target_bir_lowering=False, debug=False, enable_asserts=False)

    bankT_d = nc.dram_tensor("bankT", [D, N], F32, kind="ExternalInput")
    qT_d = nc.dram_tensor("qT", [D, S], F32, kind="ExternalInput")
    sqm_d = nc.dram_tensor("sqmrow", [1, S], F32, kind="ExternalInput")
    sqn_d = nc.dram_tensor("sqncol", [128, N // 128], F32, kind="ExternalInput")
    out_d = nc.dram_tensor("out", [N, S], F32, kind="ExternalOutput")

    with tile.TileContext(nc) as tc:
        with (
            tc.tile_pool(name="persist", bufs=1) as persist,
            tc.tile_pool(name="psum", bufs=3, space=bass.MemorySpace.PSUM) as psum_pool,
            tc.tile_pool(name="stage", bufs=3) as stage,
            tc.tile_pool(name="outp", bufs=3) as outp,
        ):
            qt = persist.tile([D, S], F32)
            qtr = persist.tile([D, S], F32R)
            nc.sync.dma_start(qt[:], qT_d.ap()[:])
            nc.vector.tensor_copy(qtr[:], qt[:])
            sqm = persist.tile([1, S], F32)
            sqmr = persist.tile([1, S], F32R)
            nc.sync.dma_start(sqm[:], sqm_d.ap()[:])
            nc.vector.tensor_copy(sqmr[:], sqm[:])
            sqn = persist.tile([128, NB], F32)
            nc.sync.dma_start(sqn[:], sqn_d.ap()[:])
            ones = persist.tile([1, 128], F32)
            onesr = persist.tile([1, 128], F32R)
            nc.vector.memset(ones[:], 1.0)
            nc.vector.tensor_copy(onesr[:], ones[:])

            bank = persist.tile([D, N], F32)
            bankr = persist.tile([D, N], F32R)
            for k in range(8):
                cs = slice(k * 1024, (k + 1) * 1024)
                nc.sync.dma_start(bank[:, cs], bankT_d.ap()[:, cs])
                nc.vector.tensor_copy(bankr[:, cs], bank[:, cs])

            for nb in range(NB):
                ps = psum_pool.tile([128, S], F32)
                for j in range(2):
                    nc.tensor.matmul(
                        ps[:, j * 512 : (j + 1) * 512],
                        bankr[:, nb * 128 : (nb + 1) * 128],
                        qtr[:, j * 512 : (j + 1) * 512],
                        start=True,
                        stop=False,
                    )
                for j in range(2):
                    nc.tensor.matmul(
                        ps[:, j * 512 : (j + 1) * 512],
                        onesr[:],
                        sqmr[:, j * 512 : (j + 1) * 512],
                        start=False,
                        stop=True,
                    )
                st = stage.tile([128, S], F32)
                nc.scalar.activation(
                    st[:],
                    ps[:],
                    mybir.ActivationFunctionType.Sqrt,
                    bias=sqn[:, nb : nb + 1],
                    scale=-2.0,
                )
                ot = outp.tile([128, S], F32)
                nc.vector.tensor_scalar_mul(ot[:], st[:], -1.0)
                nc.sync.dma_start(out_d.ap()[nb * 128 : (nb + 1) * 128, :], ot[:])

    nc.compile()
    return nc


def _build():
    if VARIANT == "devsq2":
        return _build_devsq2()
    return _build_hostsq()


def _prep_in_maps(feats):
    featT = np.ascontiguousarray(feats.T)
    in_maps = []
    if VARIANT == "devsq2":
        for c in range(NCORES):
            sl = slice(c * S, (c + 1) * S)
            in_maps.append({"bankT": featT, "qT": np.ascontiguousarray(featT[:, sl])})
        return in_maps
    sq = np.sum(feats.astype(np.float64) * feats.astype(np.float64), axis=1).astype(
        np.float32
    )
    sqncol = np.ascontiguousarray((sq + EPS).reshape(NB, 128).T)
    for c in range(NCORES):
        sl = slice(c * S, (c + 1) * S)
        in_maps.append(
            {
                "bankT": featT,
                "qT": np.ascontiguousarray(featT[:, sl]),
                "sqmrow": np.ascontiguousarray((-0.5 * sq[sl]).reshape(1, S)),
                "sqncol": sqncol,
            }
        )
    return in_maps


def kernel(features):
    global LAST_RESULTS
    feats = np.ascontiguousarray(np.asarray(features), dtype=np.float32)
    assert feats.shape == (N, D)

    if "nc" not in _STATE:
        _STATE["nc"] = _build()
    nc = _STATE["nc"]

    in_maps = _prep_in_maps(feats)
    res = run_bass_kernel_spmd(nc, in_maps, list(range(NCORES)))
    LAST_RESULTS = res

    out = np.concatenate([res.results[c]["out"] for c in range(NCORES)], axis=1)
    np.fill_diagonal(out, -0.0)
    return out


def bench(features, iters=24, warmup=4):
    """Estimate device exec time per kernel invocation.

    No NTFF profiling hooks exist in this container, so measure by
    dispatching the compiled shard_map executable repeatedly with the
    previous outputs donated as the next call's output buffers (all data
    stays on device) and timing the marginal cost per dispatch.
    """
    import time

    import jax
    from jax.sharding import Mesh, NamedSharding, PartitionSpec
    from jax.experimental.shard_map import shard_map

    from concourse import bass2jax

    feats = np.ascontiguousarray(np.asarray(features), dtype=np.float32)
    if "nc" not in _STATE:
        _STATE["nc"] = _build()
    nc = _STATE["nc"]
    in_maps = _prep_in_maps(feats)

    bass2jax.install_neuronx_cc_hook()

    import concourse.mybir as mb

    partition_name = nc.partition_id_tensor.name if nc.partition_id_tensor else None
    in_names, out_names, out_avals, zero_outs = [], [], [], []
    for alloc in nc.m.functions[0].allocations:
        if not isinstance(alloc, mb.MemoryLocationSet):
            continue
        name = alloc.memorylocations[0].name
        if alloc.kind == "ExternalInput":
            if name != partition_name:
                in_names.append(name)
        elif alloc.kind == "ExternalOutput":
            out_names.append(name)
            shape = tuple(alloc.tensor_shape)
            dtype = mb.dt.np(alloc.dtype)
            out_avals.append(jax.core.ShapedArray(shape, dtype))
            zero_outs.append(np.zeros(shape, dtype))
    n_params = len(in_names)
    all_names = in_names + out_names

    if partition_name is not None:
        all_names = all_names + [partition_name]

    def _body(*args):
        operands = list(args)
        if partition_name is not None:
            operands.append(bass2jax.partition_id_tensor())
        outs = bass2jax._bass_exec_p.bind(
            *operands,
            out_avals=tuple(out_avals),
            in_names=tuple(all_names),
            out_names=tuple(out_names),
            lowering_input_output_aliases=(),
            sim_require_finite=True,
            sim_require_nnan=True,
            nc=nc,
        )
        return tuple(outs)

    devices = jax.devices()[:NCORES]
    mesh = Mesh(np.asarray(devices), ("core",))
    nout = len(out_names)
    donate = tuple(range(n_params, n_params + nout))
    f = jax.jit(
        shard_map(
            _body,
            mesh=mesh,
            in_specs=(PartitionSpec("core"),) * (n_params + nout),
            out_specs=(PartitionSpec("core"),) * nout,
            check_rep=False,
        ),
        donate_argnums=donate,
        keep_unused=True,
    )

    sharding = NamedSharding(mesh, PartitionSpec("core"))
    ins_dev = [
        jax.device_put(
            np.concatenate([in_maps[c][name] for c in range(NCORES)], axis=0), sharding
        )
        for name in in_names
    ]
    outs = tuple(
        jax.device_put(np.zeros((NCORES * z.shape[0], *z.shape[1:]), z.dtype), sharding)
        for z in zero_outs
    )

    for _ in range(warmup):
        outs = f(*ins_dev, *outs)
    jax.block_until_ready(outs)

    t0 = time.perf_counter()
    for _ in range(iters):
        outs = f(*ins_dev, *outs)
    jax.block_until_ready(outs)
    t1 = time.perf_counter()
    return (t1 - t0) / iters * 1e9


# revision 37
# speedup vs baseline: 52.5127x; 52.5127x over previous
"""Trainium2 Bass kernel for FeatureSimilarity (l2): out = -||f_i - f_j|| over all pairs.

Default strategy ("tri", 8 NeuronCores, SPMD): the 8192x8192 output is
symmetric, so only the 136 unique 512x512 cells of its 16x16 block grid
(lower triangle + diagonal) are computed -- 17 cells per core -- cutting HBM
writes from 32 MiB to 17 MiB per core.  With all 8 cores running, core pairs
share an HBM stack (~190 GB/s effective per core), so HBM writes are the
roofline and this is ~2.3x faster than computing full row slabs.

Per core (fully static, uniform program; per-core data packed on the host):
  inputs  rowpack/colpack [128, 17*512]: transposed features of each cell's
          row-block / column-block, concatenated in cell order.
  output  outpack [17*512, 512]: the 17 cells stacked (contiguous writes).

  Per 512-column chunk i (prologue, interleaved with main tiles at lag 2 so
  the in-order PE stream never parks behind the input DMA):
    round-copy to float32r (BIR requires fp32r matmul operands to be
    produced rounded), square the ROUNDED values (keeps the diagonal
    cancellation tight), then PE-reduce:
      sq_col row [1, 512] via a [-0.5]-column x csq matmul,
      sq_row cols [128, 4] via 4 N=1 matmuls against a ones column
    (each into its OWN psum pool tile, read full-range -- disjoint-range
    sharing would let PE writes overlap DVE reads in one PSUM bank, which
    is fatal on TRN2).
  Per 128x512 main tile (4 per cell):
    PSUM = rowblk_r^T @ colcell_r - 0.5*sq_col[j]   (fp32r matmul + K=1
           augmented matmul with a rounded ones-row)
    SBUF = Sqrt(-2*PSUM + (sq_row[p] + EPS))        (ACT, bias AP)
    out  = SBUF * -1                                (DVE)
    DMA to outpack.
  Host: scatter cells into the full matrix, mirror transposes for r != c,
  write the (identically zero) diagonal.
  EPS keeps the Sqrt input positive under fp32r noise (see note at EPS).
"""

import os
import sys

import numpy as np

sys.path.insert(0, "/opt/trn_rl_repo")

import concourse.bacc as bacc
import concourse.bass as bass
import concourse.mybir as mybir
import concourse.tile as tile
from concourse.bass_utils import run_bass_kernel_spmd

N = 8192
D = 128
NCORES = 8
S = N // NCORES  # 1024 columns per core
NB = N // 128  # 64 row blocks per core
# Added to d^2 so the ACT Sqrt input stays positive under fp32r matmul noise.
# Measured diagonal noise (rounded-square norms) is +/-0.031; off-diagonal
# d^2 >= ~70 for this data, so the systematic error is eps/(2*dist) <= 3e-3.
EPS = 0.0625
F32 = mybir.dt.float32
F32R = mybir.dt.float32r

VARIANT = os.environ.get("KERNEL_VARIANT", "tri")
REPS = int(os.environ.get("KERNEL_REPS", "1"))  # main-loop repetitions (benchmarking)

_STATE = {}
LAST_RESULTS = None


def _build_devsq2(reps=1):
    """Device-side norms via PE reductions; fp32r matmuls with explicit
    rounding copies on DVE (BIR requires fp32r matmul operands to be
    produced rounded)."""
    nc = bacc.Bacc("TRN2", target_bir_lowering=False, debug=False, enable_asserts=False)

    bankT_d = nc.dram_tensor("bankT", [D, N], F32, kind="ExternalInput")
    qT_d = nc.dram_tensor("qT", [D, S], F32, kind="ExternalInput")
    out_d = nc.dram_tensor("out", [N, S], F32, kind="ExternalOutput")

    CH = 8
    CW = N // CH

    with tile.TileContext(nc) as tc:
        with (
            tc.tile_pool(name="persist", bufs=1) as persist,
            tc.tile_pool(name="psum", bufs=2, space=bass.MemorySpace.PSUM) as psum_pool,
            tc.tile_pool(name="prosum", bufs=2, space=bass.MemorySpace.PSUM) as prosum,
            tc.tile_pool(name="stage", bufs=3) as stage,
            tc.tile_pool(name="outp", bufs=3) as outp,
        ):
            qt = persist.tile([D, S], F32)
            qtr = persist.tile([D, S], F32R)
            nc.sync.dma_start(qt[:], qT_d.ap()[:])
            nc.vector.tensor_copy(qtr[:], qt[:])

            bank = persist.tile([D, N], F32)
            bankr = persist.tile([D, N], F32R)
            bsq = persist.tile([D, N], F32)
            qsq = persist.tile([D, S], F32)
            sqncol = persist.tile([128, NB], F32)  # sq_n + EPS, column form
            sqm = persist.tile([1, S], F32R)  # -0.5 * sq_m, row form (rounded)
            ones = persist.tile([1, 128], F32)
            onesr = persist.tile([1, 128], F32R)  # aug lhsT (rounded)
            onescol = persist.tile([128, 1], F32)  # rhs for sq_n reduce
            neghalf = persist.tile([128, 1], F32)  # lhsT for sq_m reduce
            nc.vector.memset(ones[:], 1.0)
            nc.vector.memset(onescol[:], 1.0)
            nc.vector.memset(neghalf[:], -0.5)
            nc.vector.tensor_copy(onesr[:], ones[:])

            # query norms: qsq = qt^2; sqm[0,j] = -0.5 * sum_d qsq[d,j]
            nc.vector.tensor_tensor(qsq[:], qt[:], qt[:], mybir.AluOpType.mult)
            for j in range(2):
                pm = prosum.tile([1, 512], F32, tag="pro")
                nc.tensor.matmul(
                    pm[:],
                    neghalf[:],
                    qsq[:, j * 512 : (j + 1) * 512],
                    start=True,
                    stop=True,
                )
                nc.vector.tensor_copy(sqm[:, j * 512 : (j + 1) * 512], pm[:])

            # bank norms, chunked with the bank DMA; rounding copy for matmuls.
            # per-chunk PSUM tiles + full-range reads avoid same-bank PE-W /
            # DVE-R overlap (fatal on TRN2).
            for k in range(CH):
                cs = slice(k * CW, (k + 1) * CW)
                nc.sync.dma_start(bank[:, cs], bankT_d.ap()[:, cs])
                nc.vector.tensor_copy(bankr[:, cs], bank[:, cs])
                nc.vector.tensor_tensor(
                    bsq[:, cs], bank[:, cs], bank[:, cs], mybir.AluOpType.mult
                )
                pn = prosum.tile([128, CH], F32, tag="pro2")
                for b in range(CH):
                    col = k * CH + b
                    nc.tensor.matmul(
                        pn[:, b : b + 1],
                        bsq[:, col * 128 : (col + 1) * 128],
                        onescol[:],
                        start=True,
                        stop=True,
                    )
                nc.vector.tensor_scalar_add(
                    sqncol[:, k * CH : (k + 1) * CH], pn[:], float(EPS)
                )

            for _rep in range(reps):
                for nb in range(NB):
                    ps = psum_pool.tile([128, S], F32)
                    for j in range(2):
                        nc.tensor.matmul(
                            ps[:, j * 512 : (j + 1) * 512],
                            bankr[:, nb * 128 : (nb + 1) * 128],
                            qtr[:, j * 512 : (j + 1) * 512],
                            start=True,
                            stop=False,
                        )
                    for j in range(2):
                        nc.tensor.matmul(
                            ps[:, j * 512 : (j + 1) * 512],
                            onesr[:],
                            sqm[:, j * 512 : (j + 1) * 512],
                            start=False,
                            stop=True,
                        )
                    st = stage.tile([128, S], F32)
                    nc.scalar.activation(
                        st[:],
                        ps[:],
                        mybir.ActivationFunctionType.Sqrt,
                        bias=sqncol[:, nb : nb + 1],
                        scale=-2.0,
                    )
                    ot = outp.tile([128, S], F32)
                    nc.vector.tensor_scalar_mul(ot[:], st[:], -1.0)
                    nc.sync.dma_start(out_d.ap()[nb * 128 : (nb + 1) * 128, :], ot[:])

    nc.compile()
    return nc


def _build_hostsq():
    """v0: norms computed on host and passed as inputs."""
    nc = bacc.Bacc("TRN2", target_bir_lowering=False, debug=False, enable_asserts=False)

    bankT_d = nc.dram_tensor("bankT", [D, N], F32, kind="ExternalInput")
    qT_d = nc.dram_tensor("qT", [D, S], F32, kind="ExternalInput")
    sqm_d = nc.dram_tensor("sqmrow", [1, S], F32, kind="ExternalInput")
    sqn_d = nc.dram_tensor("sqncol", [128, N // 128], F32, kind="ExternalInput")
    out_d = nc.dram_tensor("out", [N, S], F32, kind="ExternalOutput")

    with tile.TileContext(nc) as tc:
        with (
            tc.tile_pool(name="persist", bufs=1) as persist,
            tc.tile_pool(name="psum", bufs=3, space=bass.MemorySpace.PSUM) as psum_pool,
            tc.tile_pool(name="stage", bufs=3) as stage,
            tc.tile_pool(name="outp", bufs=3) as outp,
        ):
            qt = persist.tile([D, S], F32)
            qtr = persist.tile([D, S], F32R)
            nc.sync.dma_start(qt[:], qT_d.ap()[:])
            nc.vector.tensor_copy(qtr[:], qt[:])
            sqm = persist.tile([1, S], F32)
            sqmr = persist.tile([1, S], F32R)
            nc.sync.dma_start(sqm[:], sqm_d.ap()[:])
            nc.vector.tensor_copy(sqmr[:], sqm[:])
            sqn = persist.tile([128, NB], F32)
            nc.sync.dma_start(sqn[:], sqn_d.ap()[:])
            ones = persist.tile([1, 128], F32)
            onesr = persist.tile([1, 128], F32R)
            nc.vector.memset(ones[:], 1.0)
            nc.vector.tensor_copy(onesr[:], ones[:])

            bank = persist.tile([D, N], F32)
            bankr = persist.tile([D, N], F32R)
            for k in range(8):
                cs = slice(k * 1024, (k + 1) * 1024)
                nc.sync.dma_start(bank[:, cs], bankT_d.ap()[:, cs])
                nc.vector.tensor_copy(bankr[:, cs], bank[:, cs])

            for nb in range(NB):
                ps = psum_pool.tile([128, S], F32)
                for j in range(2):
                    nc.tensor.matmul(
                        ps[:, j * 512 : (j + 1) * 512],
                        bankr[:, nb * 128 : (nb + 1) * 128],
                        qtr[:, j * 512 : (j + 1) * 512],
                        start=True,
                        stop=False,
                    )
                for j in range(2):
                    nc.tensor.matmul(
                        ps[:, j * 512 : (j + 1) * 512],
                        onesr[:],
                        sqmr[:, j * 512 : (j + 1) * 512],
                        start=False,
                        stop=True,
                    )
                st = stage.tile([128, S], F32)
                nc.scalar.activation(
                    st[:],
                    ps[:],
                    mybir.ActivationFunctionType.Sqrt,
                    bias=sqn[:, nb : nb + 1],
                    scale=-2.0,
                )
                ot = outp.tile([128, S], F32)
                nc.vector.tensor_scalar_mul(ot[:], st[:], -1.0)
                nc.sync.dma_start(out_d.ap()[nb * 128 : (nb + 1) * 128, :], ot[:])

    nc.compile()
    return nc


NCELL = 17  # unique 512x512 cells per core: (16 diag + 120 lower) / 8
CW = 512  # cell width
PACKW = NCELL * CW  # 8704


def _cell_assignment():
    """Split the 136 unique cells of the 16x16 symmetric grid across 8 cores."""
    cells = [(r, c) for r in range(16) for c in range(r + 1)]  # c <= r: lower+diag
    assert len(cells) == NCORES * NCELL
    return [cells[c::NCORES] for c in range(NCORES)]


def _build_tri(reps=1):
    """Symmetric-aware variant: each core computes 17 packed 512x512 cells of
    the lower triangle (the upper triangle is mirrored on the host), cutting
    HBM writes from 32 MiB to 17 MiB per core.  Same math per 128x512 tile as
    devsq2."""
    nc = bacc.Bacc("TRN2", target_bir_lowering=False, debug=False, enable_asserts=False)

    rowp_d = nc.dram_tensor("rowpack", [D, PACKW], F32, kind="ExternalInput")
    colp_d = nc.dram_tensor("colpack", [D, PACKW], F32, kind="ExternalInput")
    out_d = nc.dram_tensor("out", [PACKW, CW], F32, kind="ExternalOutput")

    with tile.TileContext(nc) as tc:
        with (
            tc.tile_pool(name="persist", bufs=1) as persist,
            tc.tile_pool(name="psum", bufs=4, space=bass.MemorySpace.PSUM) as psum_pool,
            tc.tile_pool(name="prosum", bufs=2, space=bass.MemorySpace.PSUM) as prosum,
            tc.tile_pool(name="stage", bufs=3) as stage,
            tc.tile_pool(name="outp", bufs=3) as outp,
        ):
            rowr = persist.tile([D, PACKW], F32R)
            colr = persist.tile([D, PACKW], F32R)
            sqrow = persist.tile([128, NCELL * 4], F32)  # sq_n + EPS per 128-block
            sqm = persist.tile([1, PACKW], F32R)  # -0.5*sq_col rows (rounded)
            ones = persist.tile([1, 128], F32)
            onesr = persist.tile([1, 128], F32R)
            onescol = persist.tile([128, 1], F32)
            neghalf = persist.tile([128, 1], F32)
            nc.vector.memset(ones[:], 1.0)
            nc.vector.memset(onescol[:], 1.0)
            nc.vector.memset(neghalf[:], -0.5)
            nc.vector.tensor_copy(onesr[:], ones[:])

            def emit_pro(i, stagein):
                cs = slice(i * CW, (i + 1) * CW)
                # column side: stage chunk, round, square, -0.5*colnorm row
                cstg = stagein.tile([D, CW], F32, tag="cstg")
                nc.sync.dma_start(cstg[:], colp_d.ap()[:, cs])
                nc.vector.tensor_copy(colr[:, cs], cstg[:])
                # square the ROUNDED values so the norms match what the fp32r
                # matmul sees -- keeps the diagonal cancellation tight
                ssq = stagein.tile([D, CW], F32, tag="ssq")
                nc.vector.tensor_tensor(
                    ssq[:], colr[:, cs], colr[:, cs], mybir.AluOpType.mult
                )
                # per-chunk PSUM tiles + full-range reads: a shared PSUM
                # accumulator with disjoint-range access would let PE writes
                # overlap DVE reads in the same bank (fatal on TRN2)
                pm = prosum.tile([1, CW], F32, tag="pro")
                nc.tensor.matmul(pm[:], neghalf[:], ssq[:], start=True, stop=True)
                nc.vector.tensor_copy(sqm[:, cs], pm[:])
                # row side: stage chunk, round, square, per-block norms
                rstg = stagein.tile([D, CW], F32, tag="rstg")
                nc.sync.dma_start(rstg[:], rowp_d.ap()[:, cs])
                nc.vector.tensor_copy(rowr[:, cs], rstg[:])
                rsq = stagein.tile([D, CW], F32, tag="rsq")
                nc.vector.tensor_tensor(
                    rsq[:], rowr[:, cs], rowr[:, cs], mybir.AluOpType.mult
                )
                pn = prosum.tile([128, 4], F32, tag="pro2")
                for b in range(4):
                    nc.tensor.matmul(
                        pn[:, b : b + 1],
                        rsq[:, b * 128 : (b + 1) * 128],
                        onescol[:],
                        start=True,
                        stop=True,
                    )
                nc.vector.tensor_scalar_add(
                    sqrow[:, i * 4 : (i + 1) * 4], pn[:], float(EPS)
                )

            def emit_main(i):
                ccs = slice(i * CW, (i + 1) * CW)
                for t in range(4):
                    blk = i * 4 + t
                    ps = psum_pool.tile([128, CW], F32)
                    nc.tensor.matmul(
                        ps[:],
                        rowr[:, blk * 128 : (blk + 1) * 128],
                        colr[:, ccs],
                        start=True,
                        stop=False,
                    )
                    nc.tensor.matmul(
                        ps[:], onesr[:], sqm[:, ccs], start=False, stop=True
                    )
                    st = stage.tile([128, CW], F32)
                    nc.scalar.activation(
                        st[:],
                        ps[:],
                        mybir.ActivationFunctionType.Sqrt,
                        bias=sqrow[:, blk : blk + 1],
                        scale=-2.0,
                    )
                    ot = outp.tile([128, CW], F32)
                    nc.vector.tensor_scalar_mul(ot[:], st[:], -1.0)
                    nc.sync.dma_start(out_d.ap()[blk * 128 : (blk + 1) * 128, :], ot[:])

            # interleave the prologue with the main tiles (lag 2 cells) so the
            # in-order PE stream is never parked behind the whole input DMA
            LAG = 2
            with tc.tile_pool(name="stagein", bufs=4) as stagein:
                for i in range(NCELL + LAG):
                    if i < NCELL:
                        emit_pro(i, stagein)
                    if i >= LAG:
                        emit_main(i - LAG)
            for _rep in range(1, reps):
                for i in range(NCELL):
                    emit_main(i)

    nc.compile()
    return nc


def _build(reps=1):
    if VARIANT == "devsq2":
        return _build_devsq2(reps)
    if VARIANT == "tri":
        return _build_tri(reps)
    return _build_hostsq()


def _prep_in_maps(feats):
    featT = np.ascontiguousarray(feats.T)
    in_maps = []
    if VARIANT == "tri":
        for cells in _cell_assignment():
            rowpack = np.concatenate(
                [featT[:, r * CW : (r + 1) * CW] for (r, c) in cells], axis=1
            )
            colpack = np.concatenate(
                [featT[:, c * CW : (c + 1) * CW] for (r, c) in cells], axis=1
            )
            in_maps.append(
                {
                    "rowpack": np.ascontiguousarray(rowpack),
                    "colpack": np.ascontiguousarray(colpack),
                }
            )
        return in_maps
    if VARIANT == "devsq2":
        for c in range(NCORES):
            sl = slice(c * S, (c + 1) * S)
            in_maps.append({"bankT": featT, "qT": np.ascontiguousarray(featT[:, sl])})
        return in_maps
    sq = np.sum(feats.astype(np.float64) * feats.astype(np.float64), axis=1).astype(
        np.float32
    )
    sqncol = np.ascontiguousarray((sq + EPS).reshape(NB, 128).T)
    for c in range(NCORES):
        sl = slice(c * S, (c + 1) * S)
        in_maps.append(
            {
                "bankT": featT,
                "qT": np.ascontiguousarray(featT[:, sl]),
                "sqmrow": np.ascontiguousarray((-0.5 * sq[sl]).reshape(1, S)),
                "sqncol": sqncol,
            }
        )
    return in_maps


def kernel(features):
    global LAST_RESULTS
    feats = np.ascontiguousarray(np.asarray(features), dtype=np.float32)
    assert feats.shape == (N, D)

    if "nc" not in _STATE:
        _STATE["nc"] = _build()
    nc = _STATE["nc"]

    in_maps = _prep_in_maps(feats)
    try:
        res = run_bass_kernel_spmd(nc, in_maps, list(range(NCORES)))
    except ModuleNotFoundError:
        # trace path unavailable (no antenv.axon_hooks in this container)
        os.environ["BASS_NEVER_TRACE"] = "1"
        res = run_bass_kernel_spmd(nc, in_maps, list(range(NCORES)))
    LAST_RESULTS = res

    if VARIANT == "tri":
        out = np.empty((N, N), dtype=np.float32)
        for core, cells in enumerate(_cell_assignment()):
            slab = res.results[core]["out"]  # [NCELL*512, 512]
            for i, (r, c) in enumerate(cells):
                blk = slab[i * CW : (i + 1) * CW, :]
                out[r * CW : (r + 1) * CW, c * CW : (c + 1) * CW] = blk
                if r != c:
                    out[c * CW : (c + 1) * CW, r * CW : (r + 1) * CW] = blk.T
    else:
        out = np.concatenate([res.results[c]["out"] for c in range(NCORES)], axis=1)
    np.fill_diagonal(out, -0.0)
    return out


def bench(features, iters=24, warmup=4, reps=None):
    """Estimate device exec time per kernel invocation.

    No NTFF profiling hooks exist in this container, so measure by
    dispatching the compiled shard_map executable repeatedly with the
    previous outputs donated as the next call's output buffers (all data
    stays on device) and timing the marginal cost per dispatch.
    """
    import time

    import jax
    from jax.sharding import Mesh, NamedSharding, PartitionSpec
    from jax.experimental.shard_map import shard_map

    from concourse import bass2jax

    feats = np.ascontiguousarray(np.asarray(features), dtype=np.float32)
    if reps is None:
        reps = REPS
    key = f"nc_r{reps}"
    if key not in _STATE:
        _STATE[key] = _build(reps)
    nc = _STATE[key]
    in_maps = _prep_in_maps(feats)

    bass2jax.install_neuronx_cc_hook()

    import concourse.mybir as mb

    partition_name = nc.partition_id_tensor.name if nc.partition_id_tensor else None
    in_names, out_names, out_avals, zero_outs = [], [], [], []
    for alloc in nc.m.functions[0].allocations:
        if not isinstance(alloc, mb.MemoryLocationSet):
            continue
        name = alloc.memorylocations[0].name
        if alloc.kind == "ExternalInput":
            if name != partition_name:
                in_names.append(name)
        elif alloc.kind == "ExternalOutput":
            out_names.append(name)
            shape = tuple(alloc.tensor_shape)
            dtype = mb.dt.np(alloc.dtype)
            out_avals.append(jax.core.ShapedArray(shape, dtype))
            zero_outs.append(np.zeros(shape, dtype))
    n_params = len(in_names)
    all_names = in_names + out_names

    if partition_name is not None:
        all_names = all_names + [partition_name]

    def _body(*args):
        operands = list(args)
        if partition_name is not None:
            operands.append(bass2jax.partition_id_tensor())
        outs = bass2jax._bass_exec_p.bind(
            *operands,
            out_avals=tuple(out_avals),
            in_names=tuple(all_names),
            out_names=tuple(out_names),
            lowering_input_output_aliases=(),
            sim_require_finite=True,
            sim_require_nnan=True,
            nc=nc,
        )
        return tuple(outs)

    dev_sel = os.environ.get("BENCH_DEVICES")
    if dev_sel:
        idxs = [int(x) for x in dev_sel.split(",")]
        devices = [jax.devices()[i] for i in idxs]
        ncores_eff = len(devices)
    else:
        devices = jax.devices()[:NCORES]
        ncores_eff = NCORES
    in_maps = in_maps[:ncores_eff]
    mesh = Mesh(np.asarray(devices), ("core",))
    nout = len(out_names)
    donate = tuple(range(n_params, n_params + nout))
    f = jax.jit(
        shard_map(
            _body,
            mesh=mesh,
            in_specs=(PartitionSpec("core"),) * (n_params + nout),
            out_specs=(PartitionSpec("core"),) * nout,
            check_rep=False,
        ),
        donate_argnums=donate,
        keep_unused=True,
    )

    sharding = NamedSharding(mesh, PartitionSpec("core"))
    ins_dev = [
        jax.device_put(
            np.concatenate([in_maps[c][name] for c in range(ncores_eff)], axis=0),
            sharding,
        )
        for name in in_names
    ]
    outs = tuple(
        jax.device_put(
            np.zeros((ncores_eff * z.shape[0], *z.shape[1:]), z.dtype), sharding
        )
        for z in zero_outs
    )

    for _ in range(warmup):
        outs = f(*ins_dev, *outs)
    jax.block_until_ready(outs)

    t0 = time.perf_counter()
    for _ in range(iters):
        outs = f(*ins_dev, *outs)
    jax.block_until_ready(outs)
    t1 = time.perf_counter()
    return (t1 - t0) / iters * 1e9
